# revision 15
# baseline (speedup 1.0000x reference)
"""GATv2 (3-layer) + pooling + MLP head on 8 TRN2 NeuronCores.

Self-contained: host-side graph partitioning (dst-sharded, 128-node groups,
128-edge tiles), fp16 device compute with fp32 accumulation, per-layer fp16
AllGather of the source-transform table, per-edge attention via indirect
gather + selection-matrix matmuls, replicated fp32 MLP head.
"""
import sys

sys.path.insert(0, "/opt/trn_rl_repo")

from contextlib import ExitStack

import numpy as np

import concourse.bass as bass
import concourse.tile as tile
from concourse import bacc, mybir
from concourse.bass_utils import run_bass_kernel_spmd

# ---- problem constants (hardcoded per task spec) ----
N_NODES = 50000
N_EDGES = 800000
N_GRAPHS = 256
F_NODE = 64
F_MOL = 200
H = 4
C = 64
HC = H * C          # 256
HCA = HC + 4        # 256 + per-head linear-attn column
NCORES = 8
P = 128
NG = 49             # groups per core
NPC = NG * P        # 6272 nodes per core
NPAD = NPC * NCORES # 50176
EPS_BN = 1e-5

F16 = mybir.dt.float16
F32 = mybir.dt.float32
I32 = mybir.dt.int32

_CACHE = {}
DBG_SKIP = set()


# ----------------------------------------------------------------------------
# host-side graph partitioning
# ----------------------------------------------------------------------------

def _prep_graph(edge_index):
    src = np.concatenate([edge_index[0], np.arange(N_NODES, dtype=np.int64)])
    dst = np.concatenate([edge_index[1], np.arange(N_NODES, dtype=np.int64)])
    order = np.argsort(dst, kind="stable")
    src, dst = src[order], dst[order]

    per_core = []
    counts = np.zeros((NCORES, NG), dtype=np.int64)
    for c in range(NCORES):
        lo, hi = c * NPC, (c + 1) * NPC
        m = (dst >= lo) & (dst < hi)
        s_c, d_c = src[m], dst[m] - lo
        g_c = d_c // P
        per_core.append((s_c, d_c, g_c))
        counts[c] = np.bincount(g_c, minlength=NG)

    g_tiles = np.maximum(1, (counts.max(axis=0) + P - 1) // P).astype(int)
    t_all = int(g_tiles.sum())

    srcidx = np.zeros((NCORES, t_all * P), dtype=np.int32)
    dcol = np.full((NCORES, t_all * P), -1.0, dtype=np.float16)
    off = 0
    for g in range(NG):
        for c in range(NCORES):
            s_c, d_c, g_c = per_core[c]
            m = g_c == g
            n = int(m.sum())
            srcidx[c, off:off + n] = s_c[m]
            dcol[c, off:off + n] = (d_c[m] - g * P).astype(np.float16)
        off += int(g_tiles[g]) * P
    # [tiles, P] -> [P, tiles]
    srcidx = srcidx.reshape(NCORES, t_all, P).transpose(0, 2, 1).copy()
    dcol = dcol.reshape(NCORES, t_all, P).transpose(0, 2, 1).copy()
    return srcidx, dcol, tuple(int(t) for t in g_tiles)


def _rep(v, rows=P):
    """replicate a row vector across partitions"""
    v = np.asarray(v).reshape(1, -1)
    return np.tile(v, (rows, 1))


def _aug_w(W, b, att):
    """[fout_in? ] W:[HC, fin], b:[HC] -> WT_aug [fin, HCA] f16, b_aug_rep f16"""
    A = np.zeros((HC, H), dtype=np.float64)
    for h in range(H):
        A[h * C:(h + 1) * C, h] = att[h]
    WT = W.T.astype(np.float64)                      # [fin, HC]
    WT_al = 0.2 * (WT @ A)                           # [fin, H]
    b_al = 0.2 * (b.astype(np.float64) @ A)          # [H]
    WT_aug = np.concatenate([WT, WT_al], axis=1).astype(np.float16)
    b_aug = np.concatenate([b, b_al]).astype(np.float32)
    return WT_aug, _rep(b_aug).astype(np.float16)


def _prep_inputs(x, edge_index, batch, mol_feats, params):
    x = np.asarray(x, dtype=np.float32)
    edge_index = np.asarray(edge_index)
    batch = np.asarray(batch)
    mol_feats = np.asarray(mol_feats, dtype=np.float32)

    srcidx, dcol, g_tiles = _prep_graph(edge_index)

    gat = [[np.asarray(t, dtype=np.float32) for t in layer]
           for layer in params["gat"]]
    bn_g, bn_b = [np.asarray(t, dtype=np.float32) for t in params["bn_gc"]]
    fcm = [[np.asarray(t, dtype=np.float32) for t in pair]
           for pair in params["fc_m"]]
    bnm_g, bnm_b = [np.asarray(t, dtype=np.float32) for t in params["bn_m"]]
    fc = [[np.asarray(t, dtype=np.float32) for t in pair]
          for pair in params["fc"]]

    xpad = np.zeros((NPAD, F_NODE), dtype=np.float32)
    xpad[:N_NODES] = x
    batch_pad = np.full(NPAD, -1, dtype=np.int64)
    batch_pad[:N_NODES] = batch

    shared = {}
    # per-layer gat weights
    for li, (Wl, bl, Wr, br, att, bias) in enumerate(gat):
        wl_aug, bl_rep = _aug_w(Wl, bl, att)
        wrT = np.ascontiguousarray(Wr.T).astype(np.float16)
        br_rep = _rep(br.astype(np.float32)).astype(np.float16)
        for k in range((wl_aug.shape[0] + P - 1) // P):
            shared[f"wl{li}k{k}"] = np.ascontiguousarray(wl_aug[k * P:(k + 1) * P])
            shared[f"wr{li}k{k}"] = np.ascontiguousarray(wrT[k * P:(k + 1) * P])
        shared[f"bl{li}"] = bl_rep
        shared[f"br{li}"] = br_rep
        shared[f"att08_{li}"] = _rep(0.8 * att.reshape(-1)).astype(np.float16)
        shared[f"bout{li}"] = _rep(bias).astype(np.float32)
    shared["use_bout"] = any(np.abs(l[5]).max() > 0 for l in gat)

    shared["iota_row"] = _rep(np.arange(P)).astype(np.float16)
    shared["iota_col"] = np.arange(P, dtype=np.float16).reshape(P, 1)
    shared["ident"] = np.eye(P, dtype=np.float16)
    shared["identf32"] = np.eye(P, dtype=np.float32)
    shared["bn_g"] = bn_g.reshape(1, -1)
    shared["bn_b"] = bn_b.reshape(1, -1)

    # mol head
    (W0m, b0m), (W1m, b1m) = fcm
    molT = np.ascontiguousarray(mol_feats.T)                 # [200, 256]
    shared["molT_a"] = molT[:128]
    shared["molT_b"] = molT[128:]                            # [72, 256]
    w0mT = np.ascontiguousarray(W0m.T)                       # [200, 64]
    shared["w0m_a"] = w0mT[:128]
    shared["w0m_b"] = w0mT[128:]
    shared["b0m"] = _rep(b0m)
    shared["w1mT"] = np.ascontiguousarray(W1m.T)             # [64, 64]
    shared["b1m"] = _rep(b1m)
    shared["bnm_g"] = bnm_g.reshape(1, -1)
    shared["bnm_b"] = bnm_b.reshape(1, -1)
    shared["ones_col"] = np.ones((P, 1), dtype=np.float32)

    # fc chain: chunked W^T
    for i, (W, b) in enumerate(fc):
        WT = np.ascontiguousarray(W.T)                       # [fin, fout]
        nk = (WT.shape[0] + P - 1) // P
        for k in range(nk):
            shared[f"fc{i}w{k}"] = WT[k * P:(k + 1) * P]
        shared[f"fc{i}b"] = _rep(b)

    in_maps = []
    for c in range(NCORES):
        lo = c * NPC
        m = {}
        m["h0T"] = np.ascontiguousarray(xpad[lo:lo + NPC].T).astype(np.float16)
        m["srcidx"] = srcidx[c]
        m["dcol"] = dcol[c]
        # BN valid mask per (row, group)
        mask = np.zeros((P, NG), dtype=np.float32)
        gids = np.arange(NPC) + lo
        mask[:, :] = (gids < N_NODES).reshape(NG, P).T
        m["maskbuf"] = mask
        # pool selection
        bc = batch_pad[lo:lo + NPC]
        g0 = int(bc[bc >= 0].min()) if (bc >= 0).any() else 0
        spool = np.zeros((NPC, P), dtype=np.float16)
        valid = bc >= 0
        assert (bc[valid] - g0).max() < P
        spool[np.nonzero(valid)[0], (bc[valid] - g0)] = 1.0
        # [NG*P, P] -> per group [P, P] laid side by side: [P, NG*P]
        m["spool"] = spool.reshape(NG, P, P).transpose(1, 0, 2).reshape(P, NG * P).copy()
        m["poolrow"] = (g0 + np.arange(P, dtype=np.int32)).reshape(P, 1)
        for k, v in shared.items():
            if k != "use_bout":
                m[k] = v
        in_maps.append(m)

    fins = [F_NODE, HC, HC]
    meta = dict(g_tiles=g_tiles, fins=fins, use_bout=bool(shared["use_bout"]))
    return in_maps, meta


# ----------------------------------------------------------------------------
# device program
# ----------------------------------------------------------------------------

def _build_program(meta):
    g_tiles = meta["g_tiles"]
    fins = meta["fins"]
    t_all = sum(g_tiles)

    nc = bacc.Bacc("TRN2", target_bir_lowering=False, debug=False,
                   num_devices=NCORES)

    def din(name, shape, dt):
        return nc.dram_tensor(name, list(shape), dt, kind="ExternalInput").ap()

    io = {}
    io["h0T"] = din("h0T", [F_NODE, NPC], F16)
    io["srcidx"] = din("srcidx", [P, t_all], I32)
    io["dcol"] = din("dcol", [P, t_all], F16)
    io["maskbuf"] = din("maskbuf", [P, NG], F32)
    io["spool"] = din("spool", [P, NG * P], F16)
    io["poolrow"] = din("poolrow", [P, 1], I32)
    for li in range(3):
        for k in range((fins[li] + P - 1) // P):
            rows = min(P, fins[li] - k * P)
            io[f"wl{li}k{k}"] = din(f"wl{li}k{k}", [rows, HCA], F16)
            io[f"wr{li}k{k}"] = din(f"wr{li}k{k}", [rows, HC], F16)
        io[f"bl{li}"] = din(f"bl{li}", [P, HCA], F16)
        io[f"br{li}"] = din(f"br{li}", [P, HC], F16)
        io[f"att08_{li}"] = din(f"att08_{li}", [P, HC], F16)
        io[f"bout{li}"] = din(f"bout{li}", [P, HC], F32)
    io["iota_row"] = din("iota_row", [P, P], F16)
    io["iota_col"] = din("iota_col", [P, 1], F16)
    io["ident"] = din("ident", [P, P], F16)
    io["identf32"] = din("identf32", [P, P], F32)
    io["bn_g"] = din("bn_g", [1, HC], F32)
    io["bn_b"] = din("bn_b", [1, HC], F32)
    io["molT_a"] = din("molT_a", [P, N_GRAPHS], F32)
    io["molT_b"] = din("molT_b", [F_MOL - P, N_GRAPHS], F32)
    io["w0m_a"] = din("w0m_a", [P, C], F32)
    io["w0m_b"] = din("w0m_b", [F_MOL - P, C], F32)
    io["b0m"] = din("b0m", [P, C], F32)
    io["w1mT"] = din("w1mT", [C, C], F32)
    io["b1m"] = din("b1m", [P, C], F32)
    io["bnm_g"] = din("bnm_g", [1, C], F32)
    io["bnm_b"] = din("bnm_b", [1, C], F32)
    io["ones_col"] = din("ones_col", [P, 1], F32)
    fc_dims = [(HC + C, 512), (512, 512), (512, 256), (256, 1)]
    for i, (fin, fout) in enumerate(fc_dims):
        nk = (fin + P - 1) // P
        for k in range(nk):
            rows = min(P, fin - k * P)
            io[f"fc{i}w{k}"] = din(f"fc{i}w{k}", [rows, fout], F32)
        io[f"fc{i}b"] = din(f"fc{i}b", [P, fout], F32)
    out_ap = nc.dram_tensor("out", [N_GRAPHS, 1], F32, kind="ExternalOutput").ap()

    with tile.TileContext(nc) as tc:
        ctx = ExitStack()
        with ctx:
            _emit(tc, nc, ctx, io, out_ap, meta)
    nc.compile()
    return nc


def _emit(tc, nc, ctx, io, out_ap, meta):
    g_tiles = meta["g_tiles"]
    fins = meta["fins"]
    use_bout = meta["use_bout"]
    t_all = sum(g_tiles)
    rg = [list(range(NCORES))]

    res = ctx.enter_context(tc.tile_pool(name="res", bufs=1))
    wk = ctx.enter_context(tc.tile_pool(name="wk", bufs=4))
    hd = ctx.enter_context(tc.tile_pool(name="hd", bufs=1))
    mm = ctx.enter_context(tc.tile_pool(name="mm", bufs=2, space="PSUM"))
    acc = ctx.enter_context(tc.tile_pool(name="acc", bufs=2, space="PSUM"))
    hold = ctx.enter_context(tc.tile_pool(name="hold", bufs=1, space="PSUM"))
    dram = ctx.enter_context(tc.tile_pool(name="dram", bufs=1, space="DRAM"))

    def load(name, dt=None, pool=res):
        ap = io[name]
        t = pool.tile(list(ap.shape), dt or ap.dtype, name=f"L_{name}")
        nc.sync.dma_start(t[:], ap[:])
        return t

    # ---- residents ----
    srcb = load("srcidx")
    dcolb = load("dcol")
    maskb = load("maskbuf")
    iota_row = load("iota_row")
    iota_col = load("iota_col")
    ident = load("ident")
    identf32 = load("identf32")
    h0T = load("h0T")
    wts = {}
    for li in range(3):
        for k in range((fins[li] + P - 1) // P):
            wts[f"wl{li}k{k}"] = load(f"wl{li}k{k}")
            wts[f"wr{li}k{k}"] = load(f"wr{li}k{k}")
        for nm in (f"bl{li}", f"br{li}", f"att08_{li}"):
            wts[nm] = load(nm)
        if use_bout:
            wts[f"bout{li}"] = load(f"bout{li}")

    xr_buf = res.tile([P, NG * HC], F16, name="xr_buf")
    hT_buf = [res.tile([P, NPC], F16, name=f"hT{k}") for k in range(2)]

    ag_in = dram.tile([NPC, HCA], F16, name="ag_in")
    _shr = "Local" if "nocoll" in DBG_SKIP else "Shared"
    tables = [dram.tile([NPAD, HCA], F16, name=f"table{li}", addr_space=_shr)
              for li in range(3)]
    bn_in = dram.tile([2, HC], F32, name="bn_in")
    bn_out = dram.tile([2, HC], F32, name="bn_out", addr_space=_shr)
    outb = dram.tile([NPC, HC], F32, name="outb")
    pool_in = dram.tile([3 * P, HC], F32, name="pool_in")
    pool_out = dram.tile([3 * P, HC], F32, name="pool_out", addr_space=_shr)

    # ---------------- helpers ----------------
    def silu_(dst, src, pool=None):
        """dst = src * sigmoid(src)"""
        s = src[:]
        d = dst[:]
        sg = (pool or wk).tile([P, s.shape[1]], F32, name="sg", tag="sg")
        nc.scalar.activation(sg[:], s, mybir.ActivationFunctionType.Sigmoid)
        nc.vector.tensor_tensor(out=d, in0=s, in1=sg[:],
                                op=mybir.AluOpType.mult)

    def transpose128(dst, src_ap, n_out_part=P, n_in_part=P, f32=False):
        """dst[j, i] = src[i, j] via PE; src [n_in_part, n_out_part]"""
        tp = mm.tile([P, P], F32 if f32 else F16, name="mm_t", tag="mmshared")
        nc.tensor.transpose(out=tp[:n_out_part, :n_in_part], in_=src_ap,
                            identity=(identf32 if f32 else ident)[:])
        nc.scalar.copy(dst, tp[:n_out_part, :n_in_part])

    # ---------------- GAT layers ----------------
    for li in range(3):
        fin = fins[li]
        nkc = (fin + P - 1) // P  # weight K chunks
        bl, br = wts[f"bl{li}"], wts[f"br{li}"]
        att08 = wts[f"att08_{li}"]

        # ---- node phase: xl/xr = h @ W^T + b ----
        for g in range(NG):
            for side in range(2):
                b = bl if side == 0 else br
                wid = HCA if side == 0 else HC
                ps = mm.tile([P, 512], F32, name="mm_n", tag="mmshared")
                for k in range(nkc):
                    rows = min(P, fin - k * P)
                    w = wts[f"wl{li}k{k}"] if side == 0 else wts[f"wr{li}k{k}"]
                    if li == 0:
                        lhs = h0T[k * P:k * P + rows, g * P:(g + 1) * P]
                    else:
                        lhs = hT_buf[k][:rows, g * P:(g + 1) * P]
                    nc.tensor.matmul(ps[:, :wid], lhsT=lhs, rhs=w[:rows, :],
                                     start=(k == 0), stop=(k == nkc - 1))
                if side == 0:
                    xt = wk.tile([P, HCA], F16, name="xt")
                    nc.vector.tensor_tensor(out=xt[:], in0=ps[:, :HCA], in1=b[:],
                                            op=mybir.AluOpType.add)
                    nc.sync.dma_start(ag_in[g * P:(g + 1) * P, :], xt[:])
                else:
                    nc.vector.tensor_tensor(
                        out=xr_buf[:, g * HC:(g + 1) * HC], in0=ps[:, :HC],
                        in1=b[:], op=mybir.AluOpType.add)

        # ---- all-gather the table ----
        table = tables[li]
        if "nocoll" in DBG_SKIP:
            nc.sync.dma_start(table[0:NPC, :], ag_in[:])
        else:
            nc.gpsimd.collective_compute(
                "AllGather", mybir.AluOpType.bypass, replica_groups=rg,
                ins=[ag_in[:].opt()], outs=[table[:].opt()])

        # ---- edge phase ----
        if li == 2:
            spoolb = load("spool")
            pacc = hold.tile([P, HC], F32, name="pacc", space="PSUM")
        t0 = 0
        for g in range(NG):
            nt = g_tiles[g]
            gacc = acc.tile([P, HCA], F32, name="gacc", space="PSUM")
            t = 0
            while t < nt:
                B = 2 if t + 1 < nt else 1
                ti = t0 + t
                BW = B * HC
                # gathers into [P, B, HCA]
                G = wk.tile([P, 2 * HCA], F16, name="Gt")
                G3 = G[:].rearrange("p (b w) -> p b w", w=HCA)
                for bi in range(B):
                    nc.gpsimd.indirect_dma_start(
                        out=G3[:, bi, :], out_offset=None, in_=table[:],
                        in_offset=bass.IndirectOffsetOnAxis(
                            ap=srcb[:, ti + bi:ti + bi + 1], axis=0))
                # selection matrices (batched is_equal)
                ST = wk.tile([P, 2 * P], F16, name="ST")
                nc.vector.tensor_tensor(
                    out=ST[:, :B * P].rearrange("p (b n) -> p b n", b=B),
                    in0=dcolb[:, ti:ti + B][:, :, None].to_broadcast([P, B, P]),
                    in1=iota_row[:, None, :].to_broadcast([P, B, P]),
                    op=mybir.AluOpType.is_equal)
                S = wk.tile([P, 2 * P], F16, name="St")
                for bi in range(B):
                    Sp = mm.tile([P, P], F16, name="mm_s", tag="mmshared")
                    nc.tensor.transpose(out=Sp[:], in_=ST[:, bi * P:(bi + 1) * P],
                                        identity=ident[:])
                    nc.scalar.copy(S[:, bi * P:(bi + 1) * P], Sp[:])
                # Z = G_xl + S @ xr  accumulated in PSUM via identity-matmul
                zps = mm.tile([P, 512], F32, name="mm_z", tag="mmshared")
                for bi in range(B):
                    nc.tensor.matmul(zps[:, bi * HC:(bi + 1) * HC],
                                     lhsT=ident[:], rhs=G3[:, bi, :HC],
                                     start=True, stop=False)
                    nc.tensor.matmul(zps[:, bi * HC:(bi + 1) * HC],
                                     lhsT=S[:, bi * P:(bi + 1) * P],
                                     rhs=xr_buf[:, g * HC:(g + 1) * HC],
                                     start=False, stop=True)
                # r = relu(Z) from PSUM
                r = wk.tile([P, 2 * HC], F16, name="rt")
                nc.scalar.activation(r[:, :BW], zps[:, :BW],
                                     mybir.ActivationFunctionType.Relu)
                # ra = r * att08 ; lsum = reduce per head
                ra = wk.tile([P, 2 * HC], F16, name="rat")
                nc.vector.tensor_tensor(
                    out=ra[:, :BW].rearrange("p (b h c) -> p b h c", b=B, h=H),
                    in0=r[:, :BW].rearrange("p (b h c) -> p b h c", b=B, h=H),
                    in1=att08[:].rearrange("p (h c) -> p h c", h=H)[:, None, :, :]
                        .to_broadcast([P, B, H, C]),
                    op=mybir.AluOpType.mult)
                lsum = wk.tile([P, 2 * H], F32, name="lsum")
                nc.vector.tensor_reduce(
                    out=lsum[:, :B * H],
                    in_=ra[:, :BW].rearrange("p (bh c) -> p bh c", c=C),
                    axis=mybir.AxisListType.X, op=mybir.AluOpType.add)
                # logit = lsum + 0.2*al[src] (table cols 256:260)
                logit = wk.tile([P, 2 * H], F32, name="logit")
                nc.vector.tensor_tensor(
                    out=logit[:, :B * H].rearrange("p (b h) -> p b h", b=B),
                    in0=lsum[:, :B * H].rearrange("p (b h) -> p b h", b=B),
                    in1=G3[:, :B, HC:HCA], op=mybir.AluOpType.add)
                # p = exp(logit) -> pay[:, :, 256:260]; pay = [G*p | p]
                pay = wk.tile([P, 2 * HCA], F16, name="pay")
                pay3 = pay[:].rearrange("p (b w) -> p b w", w=HCA)
                nc.scalar.activation(
                    pay3[:, :B, HC:HCA],
                    logit[:, :B * H].rearrange("p (b h) -> p b h", b=B),
                    mybir.ActivationFunctionType.Exp)
                nc.vector.tensor_tensor(
                    out=pay3[:, :B, :HC].rearrange("p b (h c) -> p b h c", h=H),
                    in0=G3[:, :B, :HC].rearrange("p b (h c) -> p b h c", h=H),
                    in1=pay3[:, :B, HC:HCA][:, :, :, None].to_broadcast([P, B, H, C]),
                    op=mybir.AluOpType.mult)
                # scatter-accumulate into group PSUM
                for bi in range(B):
                    nc.tensor.matmul(gacc[:], lhsT=ST[:, bi * P:(bi + 1) * P],
                                     rhs=pay3[:, bi, :],
                                     start=(t + bi == 0), stop=(t + bi == nt - 1))
                t += B
            t0 += nt

            # ---- group finalize ----
            den = wk.tile([P, H], F32, name="den")
            nc.vector.tensor_scalar_add(den[:], gacc[:, HC:HCA], 1e-16)
            rec = wk.tile([P, H], F32, name="rec")
            nc.vector.reciprocal(rec[:], den[:])
            ogt = wk.tile([P, HC], F32, name="ogt")
            og = ogt[:, :]
            nc.vector.tensor_tensor(
                out=og.rearrange("p (h c) -> p h c", h=H),
                in0=gacc[:, :HC].rearrange("p (h c) -> p h c", h=H),
                in1=rec[:, :, None].to_broadcast([P, H, C]),
                op=mybir.AluOpType.mult)
            if use_bout:
                nc.vector.tensor_tensor(out=og, in0=og, in1=wts[f"bout{li}"][:],
                                        op=mybir.AluOpType.add)
            if li == 0:
                nc.sync.dma_start(outb[g * P:(g + 1) * P, :], ogt[:])
            elif li == 1:
                hg16 = wk.tile([P, HC], F16, name="hg16b")
                silu_(hg16, ogt)
                for k in range(2):
                    transpose128(hT_buf[k][:, g * P:(g + 1) * P],
                                 hg16[:, k * P:(k + 1) * P])
            else:
                hg16 = wk.tile([P, HC], F16, name="hg16c")
                silu_(hg16, ogt)
                nc.tensor.matmul(pacc[:], lhsT=spoolb[:, g * P:(g + 1) * P],
                                 rhs=hg16[:], start=(g == 0), stop=(g == NG - 1))

        # ---- layer activation ----
        if li == 0:
            # BN stats: s1/s2 via mask-weighted column-sum matmuls
            stats = hold.tile([33, HC], F32, name="stats", space="PSUM")
            for g in range(NG):
                ld = wk.tile([P, HC], F32, name="ld1")
                nc.sync.dma_start(ld[:], outb[g * P:(g + 1) * P, :])
                sq = wk.tile([P, HC], F32, name="sq")
                nc.scalar.activation(sq[:], ld[:], mybir.ActivationFunctionType.Square)
                nc.tensor.matmul(stats[0:1, :], lhsT=maskb[:, g:g + 1], rhs=ld[:],
                                 start=(g == 0), stop=(g == NG - 1))
                nc.tensor.matmul(stats[32:33, :], lhsT=maskb[:, g:g + 1], rhs=sq[:],
                                 start=(g == 0), stop=(g == NG - 1))
            s1_sb = hd.tile([1, HC], F32, name="s1_sb")
            s2_sb = hd.tile([1, HC], F32, name="s2_sb")
            nc.scalar.copy(s1_sb[:], stats[0:1, :])
            nc.scalar.copy(s2_sb[:], stats[32:33, :])
            nc.sync.dma_start(bn_in[0:1, :], s1_sb[:])
            nc.sync.dma_start(bn_in[1:2, :], s2_sb[:])
            nc.gpsimd.collective_compute(
                "AllReduce", mybir.AluOpType.add, replica_groups=rg,
                ins=[bn_in[:].opt()], outs=[bn_out[:].opt()])
            s1t = hd.tile([1, HC], F32, name="s1t")
            s2t = hd.tile([1, HC], F32, name="s2t")
            nc.sync.dma_start(s1t[:], bn_out[0:1, :])
            nc.sync.dma_start(s2t[:], bn_out[1:2, :])
            gam = load("bn_g", pool=hd)
            bet = load("bn_b", pool=hd)
            mu = hd.tile([1, HC], F32, name="mu")
            nc.vector.tensor_scalar_mul(mu[:], s1t[:], 1.0 / N_NODES)
            mu2 = hd.tile([1, HC], F32, name="mu2")
            nc.scalar.activation(mu2[:], mu[:], mybir.ActivationFunctionType.Square)
            var = hd.tile([1, HC], F32, name="var")
            nc.vector.tensor_scalar(var[:], s2t[:], 1.0 / N_NODES, None,
                                    op0=mybir.AluOpType.mult)
            nc.vector.tensor_tensor(out=var[:], in0=var[:], in1=mu2[:],
                                    op=mybir.AluOpType.subtract)
            nc.vector.tensor_scalar_add(var[:], var[:], EPS_BN)
            sd = hd.tile([1, HC], F32, name="sd")
            nc.scalar.activation(sd[:], var[:], mybir.ActivationFunctionType.Sqrt)
            rsd = hd.tile([1, HC], F32, name="rsd")
            nc.vector.reciprocal(rsd[:], sd[:])
            ge = hd.tile([1, HC], F32, name="ge")
            nc.vector.tensor_tensor(out=ge[:], in0=gam[:], in1=rsd[:],
                                    op=mybir.AluOpType.mult)
            muge = hd.tile([1, HC], F32, name="muge")
            nc.vector.tensor_tensor(out=muge[:], in0=mu[:], in1=ge[:],
                                    op=mybir.AluOpType.mult)
            be = hd.tile([1, HC], F32, name="be")
            nc.vector.tensor_tensor(out=be[:], in0=bet[:], in1=muge[:],
                                    op=mybir.AluOpType.subtract)
            ge_b = res.tile([P, HC], F32, name="ge_b")
            be_b = res.tile([P, HC], F32, name="be_b")
            if "pbcast" not in DBG_SKIP:
                nc.gpsimd.partition_broadcast(ge_b[:], ge[:])
                nc.gpsimd.partition_broadcast(be_b[:], be[:])
            else:
                nc.gpsimd.memset(ge_b[:], 1.0)
                nc.gpsimd.memset(be_b[:], 0.0)
            # second pass: h = silu(out*ge + be), transpose into hT
            for g in range(NG):
                ld = wk.tile([P, HC], F32, name="ld2")
                nc.sync.dma_start(ld[:], outb[g * P:(g + 1) * P, :])
                t1 = wk.tile([P, HC], F32, name="t1")
                nc.vector.tensor_tensor(out=t1[:], in0=ld[:], in1=ge_b[:],
                                        op=mybir.AluOpType.mult)
                nc.vector.tensor_tensor(out=t1[:], in0=t1[:], in1=be_b[:],
                                        op=mybir.AluOpType.add)
                hg16 = wk.tile([P, HC], F16, name="hg16")
                silu_(hg16, t1)
                for k in range(2):
                    transpose128(hT_buf[k][:, g * P:(g + 1) * P],
                                 hg16[:, k * P:(k + 1) * P])
        elif li == 2:
            prow = load("poolrow", pool=hd)
            psb = hd.tile([P, HC], F32, name="psb")
            nc.scalar.copy(psb[:], pacc[:])
            zz = hd.tile([P, HC], F32, name="zz")
            nc.gpsimd.memset(zz[:], 0.0)
            for k in range(3):
                nc.sync.dma_start(pool_in[k * P:(k + 1) * P, :], zz[:])
            nc.gpsimd.indirect_dma_start(
                out=pool_in[:], out_offset=bass.IndirectOffsetOnAxis(
                    ap=prow[:, :1], axis=0),
                in_=psb[:], in_offset=None)
            if "nocoll" in DBG_SKIP:
                ptmp = wk.tile([P, HC], F32, name="ptmp")
                for kk in range(3):
                    nc.sync.dma_start(ptmp[:], pool_in[kk * P:(kk + 1) * P, :])
                    nc.sync.dma_start(pool_out[kk * P:(kk + 1) * P, :], ptmp[:])
            else:
                nc.gpsimd.collective_compute(
                    "AllReduce", mybir.AluOpType.add, replica_groups=rg,
                    ins=[pool_in[:].opt()], outs=[pool_out[:].opt()])

    # ---------------- head (replicated fp32) ----------------
    ones = load("ones_col")

    # mol path: hm1 = mol @ W0m^T + b0m   [256, 64]
    molT = [load("molT_a"), load("molT_b")]
    w0m = [load("w0m_a"), load("w0m_b")]
    b0m = load("b0m", pool=hd)
    hm1 = [hd.tile([P, C], F32, name=f"hm1_{r}", tag="hm", bufs=6) for r in range(2)]
    for r in range(2):
        ps = mm.tile([P, 512], F32, name="mm_h", tag="mmshared")
        for k in range(2):
            nc.tensor.matmul(ps[:, :C], lhsT=molT[k][:, r * P:(r + 1) * P],
                             rhs=w0m[k][:], start=(k == 0), stop=(k == 1))
        nc.vector.tensor_tensor(out=hm1[r][:], in0=ps[:, :C], in1=b0m[:],
                                op=mybir.AluOpType.add)
    # BN over graphs
    mst = hold.tile([33, C], F32, name="mst", space="PSUM")
    for r in range(2):
        sq = wk.tile([P, C], F32, name="msq")
        nc.scalar.activation(sq[:], hm1[r][:], mybir.ActivationFunctionType.Square)
        nc.tensor.matmul(mst[0:1, :], lhsT=ones[:], rhs=hm1[r][:],
                         start=(r == 0), stop=(r == 1))
        nc.tensor.matmul(mst[32:33, :], lhsT=ones[:], rhs=sq[:],
                         start=(r == 0), stop=(r == 1))
    gm = load("bnm_g", pool=hd)
    bm = load("bnm_b", pool=hd)
    mu = hd.tile([1, C], F32, name="mmu")
    nc.vector.tensor_scalar_mul(mu[:], mst[0:1, :], 1.0 / N_GRAPHS)
    mu2 = hd.tile([1, C], F32, name="mmu2")
    nc.scalar.activation(mu2[:], mu[:], mybir.ActivationFunctionType.Square)
    var = hd.tile([1, C], F32, name="mvar")
    nc.vector.tensor_scalar(var[:], mst[32:33, :], 1.0 / N_GRAPHS, None,
                            op0=mybir.AluOpType.mult)
    nc.vector.tensor_tensor(out=var[:], in0=var[:], in1=mu2[:],
                            op=mybir.AluOpType.subtract)
    nc.vector.tensor_scalar_add(var[:], var[:], EPS_BN)
    sd = hd.tile([1, C], F32, name="msd")
    nc.scalar.activation(sd[:], var[:], mybir.ActivationFunctionType.Sqrt)
    rsd = hd.tile([1, C], F32, name="mrsd")
    nc.vector.reciprocal(rsd[:], sd[:])
    ge = hd.tile([1, C], F32, name="mge")
    nc.vector.tensor_tensor(out=ge[:], in0=gm[:], in1=rsd[:],
                            op=mybir.AluOpType.mult)
    muge = hd.tile([1, C], F32, name="mmuge")
    nc.vector.tensor_tensor(out=muge[:], in0=mu[:], in1=ge[:],
                            op=mybir.AluOpType.mult)
    be = hd.tile([1, C], F32, name="mbe")
    nc.vector.tensor_tensor(out=be[:], in0=bm[:], in1=muge[:],
                            op=mybir.AluOpType.subtract)
    ge_b = hd.tile([P, C], F32, name="mge_b")
    be_b = hd.tile([P, C], F32, name="mbe_b")
    nc.gpsimd.partition_broadcast(ge_b[:], ge[:])
    nc.gpsimd.partition_broadcast(be_b[:], be[:])
    hm2 = [hd.tile([P, C], F32, name=f"hm2_{r}", tag="hm", bufs=6) for r in range(2)]
    for r in range(2):
        nc.vector.tensor_tensor(out=hm2[r][:], in0=hm1[r][:], in1=ge_b[:],
                                op=mybir.AluOpType.mult)
        nc.vector.tensor_tensor(out=hm2[r][:], in0=hm2[r][:], in1=be_b[:],
                                op=mybir.AluOpType.add)
        silu_(hm2[r], hm2[r], pool=hd)  # f32 in-place via sigmoid+mul (dst f32)
    # hm3 = silu(hm2 @ W1m^T + b1m)
    hm2T = hd.tile([C, N_GRAPHS], F32, name="hm2T")
    for r in range(2):
        transpose128(hm2T[:, r * P:(r + 1) * P], hm2[r][:, :C], n_out_part=C,
                     f32=True)
    w1m = load("w1mT", pool=hd)
    b1m = load("b1m", pool=hd)
    hm3 = [hd.tile([P, C], F32, name=f"hm3_{r}", tag="hm", bufs=6) for r in range(2)]
    for r in range(2):
        ps = mm.tile([P, 512], F32, name="mm_h2", tag="mmshared")
        nc.tensor.matmul(ps[:, :C], lhsT=hm2T[:, r * P:(r + 1) * P], rhs=w1m[:],
                         start=True, stop=True)
        nc.vector.tensor_tensor(out=hm3[r][:], in0=ps[:, :C], in1=b1m[:],
                                op=mybir.AluOpType.add)
        silu_(hm3[r], hm3[r], pool=hd)

    # concat [hg | hm3] -> [256, 320]
    cur = [hd.tile([P, 512], F32, name=f"hgc_{r}", tag="ioh", bufs=4) for r in range(2)]
    for r in range(2):
        nc.sync.dma_start(cur[r][:, :HC], pool_out[r * P:(r + 1) * P, :])
        nc.vector.tensor_copy(cur[r][:, HC:HC + C], hm3[r][:])

    # fc chain
    fc_dims = [(HC + C, 512), (512, 512), (512, 256), (256, 1)]
    for i, (fin, fout) in enumerate(fc_dims):
        nk = (fin + P - 1) // P
        curT = [hd.tile([min(P, fin - k * P), N_GRAPHS], F32,
                        name=f"curT{i}_{k}", tag="curT", bufs=4) for k in range(nk)]
        for r in range(2):
            for k in range(nk):
                rows = min(P, fin - k * P)
                transpose128(curT[k][:, r * P:(r + 1) * P][:, :],
                             cur[r][:, k * P:k * P + rows], n_out_part=rows,
                             f32=True)
        wch = [hd.tile([io[f"fc{i}w{k}"].shape[0], io[f"fc{i}w{k}"].shape[1]], F32, name=f"Lfc{i}w{k}", tag="fcw", bufs=4) for k in range(nk)]
        for k in range(nk):
            nc.sync.dma_start(wch[k][:], io[f"fc{i}w{k}"][:])
        bch = hd.tile([P, fc_dims[i][1]], F32, name=f"Lfc{i}b", tag="fcb", bufs=2)
        nc.sync.dma_start(bch[:], io[f"fc{i}b"][:])
        nxt = [hd.tile([P, max(fout, 1)], F32, name=f"nx{i}_{r}", tag="ioh", bufs=4) for r in range(2)]
        for r in range(2):
            ps = mm.tile([P, 512], F32, name=f"mm_fc", tag="mmshared")
            for k in range(nk):
                rows = min(P, fin - k * P)
                nc.tensor.matmul(ps[:, :fout],
                                 lhsT=curT[k][:, r * P:(r + 1) * P],
                                 rhs=wch[k][:], start=(k == 0), stop=(k == nk - 1))
            nc.vector.tensor_tensor(out=nxt[r][:, :fout], in0=ps[:, :fout],
                                    in1=bch[:, :fout], op=mybir.AluOpType.add)
            if i < 3:
                silu_(nxt[r], nxt[r], pool=hd)
        cur = nxt

    for r in range(2):
        nc.sync.dma_start(out_ap[r * P:(r + 1) * P, :], cur[r][:, :1])


# ----------------------------------------------------------------------------
# public entry
# ----------------------------------------------------------------------------

def get_program_and_inputs(x, edge_index, batch, mol_feats, params):
    in_maps, meta = _prep_inputs(x, edge_index, batch, mol_feats, params)
    key = (meta["g_tiles"], meta["use_bout"])
    if key not in _CACHE:
        _CACHE[key] = _build_program(meta)
    return _CACHE[key], in_maps


def kernel(x, edge_index, batch, mol_feats, params):
    nc, in_maps = get_program_and_inputs(x, edge_index, batch, mol_feats, params)
    res = run_bass_kernel_spmd(nc, in_maps, list(range(NCORES)))
    return np.asarray(res.results[0]["out"], dtype=np.float32)


# revision 16
# speedup vs baseline: 1.6295x; 1.6295x over previous
"""GATv2 (3-layer) + pooling + MLP head on 8 TRN2 NeuronCores.

Self-contained: host-side graph partitioning (dst-sharded, 128-node groups,
128-edge tiles), fp16 device compute with fp32 accumulation, per-layer fp16
AllGather of the source-transform table, per-edge attention via indirect
gather + selection-matrix matmuls, replicated fp32 MLP head.
"""
import sys

sys.path.insert(0, "/opt/trn_rl_repo")

from contextlib import ExitStack

import numpy as np

import concourse.bass as bass
import concourse.tile as tile
from concourse import bacc, mybir
from concourse.bass_utils import run_bass_kernel_spmd

# ---- problem constants (hardcoded per task spec) ----
N_NODES = 50000
N_EDGES = 800000
N_GRAPHS = 256
F_NODE = 64
F_MOL = 200
H = 4
C = 64
HC = H * C          # 256
HCA = HC + 4        # 256 + per-head linear-attn column
NCORES = 8
P = 128
NG = 49             # groups per core
NPC = NG * P        # 6272 nodes per core
NPAD = NPC * NCORES # 50176
EPS_BN = 1e-5

F16 = mybir.dt.float16
F32 = mybir.dt.float32
I32 = mybir.dt.int32

_CACHE = {}
DBG_SKIP = set()


# ----------------------------------------------------------------------------
# host-side graph partitioning
# ----------------------------------------------------------------------------

def _prep_graph(edge_index):
    src = np.concatenate([edge_index[0], np.arange(N_NODES, dtype=np.int64)])
    dst = np.concatenate([edge_index[1], np.arange(N_NODES, dtype=np.int64)])
    order = np.argsort(dst, kind="stable")
    src, dst = src[order], dst[order]

    per_core = []
    counts = np.zeros((NCORES, NG), dtype=np.int64)
    for c in range(NCORES):
        lo, hi = c * NPC, (c + 1) * NPC
        m = (dst >= lo) & (dst < hi)
        s_c, d_c = src[m], dst[m] - lo
        g_c = d_c // P
        per_core.append((s_c, d_c, g_c))
        counts[c] = np.bincount(g_c, minlength=NG)

    g_tiles = np.maximum(1, (counts.max(axis=0) + P - 1) // P).astype(int)
    t_all = int(g_tiles.sum())

    srcidx = np.zeros((NCORES, t_all * P), dtype=np.int32)
    dcol = np.full((NCORES, t_all * P), -1.0, dtype=np.float16)
    off = 0
    for g in range(NG):
        for c in range(NCORES):
            s_c, d_c, g_c = per_core[c]
            m = g_c == g
            n = int(m.sum())
            srcidx[c, off:off + n] = s_c[m]
            dcol[c, off:off + n] = (d_c[m] - g * P).astype(np.float16)
        off += int(g_tiles[g]) * P
    # [tiles, P] -> [P, tiles]
    srcidx = srcidx.reshape(NCORES, t_all, P).transpose(0, 2, 1).copy()
    dcol = dcol.reshape(NCORES, t_all, P).transpose(0, 2, 1).copy()
    return srcidx, dcol, tuple(int(t) for t in g_tiles)


def _rep(v, rows=P):
    """replicate a row vector across partitions"""
    v = np.asarray(v).reshape(1, -1)
    return np.tile(v, (rows, 1))


def _aug_w(W, b, att):
    """[fout_in? ] W:[HC, fin], b:[HC] -> WT_aug [fin, HCA] f16, b_aug_rep f16"""
    A = np.zeros((HC, H), dtype=np.float64)
    for h in range(H):
        A[h * C:(h + 1) * C, h] = att[h]
    WT = W.T.astype(np.float64)                      # [fin, HC]
    WT_al = 0.2 * (WT @ A)                           # [fin, H]
    b_al = 0.2 * (b.astype(np.float64) @ A)          # [H]
    WT_aug = np.concatenate([WT, WT_al], axis=1).astype(np.float16)
    b_aug = np.concatenate([b, b_al]).astype(np.float32)
    return WT_aug, _rep(b_aug).astype(np.float16)


def _prep_inputs(x, edge_index, batch, mol_feats, params):
    x = np.asarray(x, dtype=np.float32)
    edge_index = np.asarray(edge_index)
    batch = np.asarray(batch)
    mol_feats = np.asarray(mol_feats, dtype=np.float32)

    srcidx, dcol, g_tiles = _prep_graph(edge_index)

    gat = [[np.asarray(t, dtype=np.float32) for t in layer]
           for layer in params["gat"]]
    bn_g, bn_b = [np.asarray(t, dtype=np.float32) for t in params["bn_gc"]]
    fcm = [[np.asarray(t, dtype=np.float32) for t in pair]
           for pair in params["fc_m"]]
    bnm_g, bnm_b = [np.asarray(t, dtype=np.float32) for t in params["bn_m"]]
    fc = [[np.asarray(t, dtype=np.float32) for t in pair]
          for pair in params["fc"]]

    xpad = np.zeros((NPAD, F_NODE), dtype=np.float32)
    xpad[:N_NODES] = x
    batch_pad = np.full(NPAD, -1, dtype=np.int64)
    batch_pad[:N_NODES] = batch

    shared = {}
    # per-layer gat weights
    for li, (Wl, bl, Wr, br, att, bias) in enumerate(gat):
        wl_aug, bl_rep = _aug_w(Wl, bl, att)
        wrT = np.ascontiguousarray(Wr.T).astype(np.float16)
        br_rep = _rep(br.astype(np.float32)).astype(np.float16)
        for k in range((wl_aug.shape[0] + P - 1) // P):
            shared[f"wl{li}k{k}"] = np.ascontiguousarray(wl_aug[k * P:(k + 1) * P])
            shared[f"wr{li}k{k}"] = np.ascontiguousarray(wrT[k * P:(k + 1) * P])
        shared[f"bl{li}"] = bl_rep
        shared[f"br{li}"] = br_rep
        shared[f"att08_{li}"] = _rep(0.8 * att.reshape(-1)).astype(np.float16)
        shared[f"bout{li}"] = _rep(bias).astype(np.float32)
    shared["use_bout"] = any(np.abs(l[5]).max() > 0 for l in gat)

    shared["iota_row"] = _rep(np.arange(P)).astype(np.float16)
    shared["iota_col"] = np.arange(P, dtype=np.float16).reshape(P, 1)
    shared["ident"] = np.eye(P, dtype=np.float16)
    shared["identf32"] = np.eye(P, dtype=np.float32)
    shared["bn_g"] = bn_g.reshape(1, -1)
    shared["bn_b"] = bn_b.reshape(1, -1)

    # mol head
    (W0m, b0m), (W1m, b1m) = fcm
    molT = np.ascontiguousarray(mol_feats.T)                 # [200, 256]
    shared["molT_a"] = molT[:128]
    shared["molT_b"] = molT[128:]                            # [72, 256]
    w0mT = np.ascontiguousarray(W0m.T)                       # [200, 64]
    shared["w0m_a"] = w0mT[:128]
    shared["w0m_b"] = w0mT[128:]
    shared["b0m"] = _rep(b0m)
    shared["w1mT"] = np.ascontiguousarray(W1m.T)             # [64, 64]
    shared["b1m"] = _rep(b1m)
    shared["bnm_g"] = bnm_g.reshape(1, -1)
    shared["bnm_b"] = bnm_b.reshape(1, -1)
    shared["ones_col"] = np.ones((P, 1), dtype=np.float32)

    # fc chain: chunked W^T
    for i, (W, b) in enumerate(fc):
        WT = np.ascontiguousarray(W.T)                       # [fin, fout]
        nk = (WT.shape[0] + P - 1) // P
        for k in range(nk):
            shared[f"fc{i}w{k}"] = WT[k * P:(k + 1) * P]
        shared[f"fc{i}b"] = _rep(b)

    in_maps = []
    for c in range(NCORES):
        lo = c * NPC
        m = {}
        m["h0T"] = np.ascontiguousarray(xpad[lo:lo + NPC].T).astype(np.float16)
        m["srcidx"] = srcidx[c]
        m["dcol"] = dcol[c]
        # BN valid mask per (row, group)
        mask = np.zeros((P, NG), dtype=np.float32)
        gids = np.arange(NPC) + lo
        mask[:, :] = (gids < N_NODES).reshape(NG, P).T
        m["maskbuf"] = mask
        # pool selection
        bc = batch_pad[lo:lo + NPC]
        g0 = int(bc[bc >= 0].min()) if (bc >= 0).any() else 0
        spool = np.zeros((NPC, P), dtype=np.float16)
        valid = bc >= 0
        assert (bc[valid] - g0).max() < P
        spool[np.nonzero(valid)[0], (bc[valid] - g0)] = 1.0
        # [NG*P, P] -> per group [P, P] laid side by side: [P, NG*P]
        m["spool"] = spool.reshape(NG, P, P).transpose(1, 0, 2).reshape(P, NG * P).copy()
        m["poolrow"] = (g0 + np.arange(P, dtype=np.int32)).reshape(P, 1)
        for k, v in shared.items():
            if k != "use_bout":
                m[k] = v
        in_maps.append(m)

    fins = [F_NODE, HC, HC]
    meta = dict(g_tiles=g_tiles, fins=fins, use_bout=bool(shared["use_bout"]))
    return in_maps, meta


# ----------------------------------------------------------------------------
# device program
# ----------------------------------------------------------------------------

def _build_program(meta):
    g_tiles = meta["g_tiles"]
    fins = meta["fins"]
    t_all = sum(g_tiles)

    nc = bacc.Bacc("TRN2", target_bir_lowering=False, debug=False,
                   num_devices=NCORES)

    def din(name, shape, dt):
        return nc.dram_tensor(name, list(shape), dt, kind="ExternalInput").ap()

    io = {}
    io["h0T"] = din("h0T", [F_NODE, NPC], F16)
    io["srcidx"] = din("srcidx", [P, t_all], I32)
    io["dcol"] = din("dcol", [P, t_all], F16)
    io["maskbuf"] = din("maskbuf", [P, NG], F32)
    io["spool"] = din("spool", [P, NG * P], F16)
    io["poolrow"] = din("poolrow", [P, 1], I32)
    for li in range(3):
        for k in range((fins[li] + P - 1) // P):
            rows = min(P, fins[li] - k * P)
            io[f"wl{li}k{k}"] = din(f"wl{li}k{k}", [rows, HCA], F16)
            io[f"wr{li}k{k}"] = din(f"wr{li}k{k}", [rows, HC], F16)
        io[f"bl{li}"] = din(f"bl{li}", [P, HCA], F16)
        io[f"br{li}"] = din(f"br{li}", [P, HC], F16)
        io[f"att08_{li}"] = din(f"att08_{li}", [P, HC], F16)
        io[f"bout{li}"] = din(f"bout{li}", [P, HC], F32)
    io["iota_row"] = din("iota_row", [P, P], F16)
    io["iota_col"] = din("iota_col", [P, 1], F16)
    io["ident"] = din("ident", [P, P], F16)
    io["identf32"] = din("identf32", [P, P], F32)
    io["bn_g"] = din("bn_g", [1, HC], F32)
    io["bn_b"] = din("bn_b", [1, HC], F32)
    io["molT_a"] = din("molT_a", [P, N_GRAPHS], F32)
    io["molT_b"] = din("molT_b", [F_MOL - P, N_GRAPHS], F32)
    io["w0m_a"] = din("w0m_a", [P, C], F32)
    io["w0m_b"] = din("w0m_b", [F_MOL - P, C], F32)
    io["b0m"] = din("b0m", [P, C], F32)
    io["w1mT"] = din("w1mT", [C, C], F32)
    io["b1m"] = din("b1m", [P, C], F32)
    io["bnm_g"] = din("bnm_g", [1, C], F32)
    io["bnm_b"] = din("bnm_b", [1, C], F32)
    io["ones_col"] = din("ones_col", [P, 1], F32)
    fc_dims = [(HC + C, 512), (512, 512), (512, 256), (256, 1)]
    for i, (fin, fout) in enumerate(fc_dims):
        nk = (fin + P - 1) // P
        for k in range(nk):
            rows = min(P, fin - k * P)
            io[f"fc{i}w{k}"] = din(f"fc{i}w{k}", [rows, fout], F32)
        io[f"fc{i}b"] = din(f"fc{i}b", [P, fout], F32)
    out_ap = nc.dram_tensor("out", [N_GRAPHS, 1], F32, kind="ExternalOutput").ap()

    with tile.TileContext(nc) as tc:
        ctx = ExitStack()
        with ctx:
            _emit(tc, nc, ctx, io, out_ap, meta)
    nc.compile()
    return nc


def _emit(tc, nc, ctx, io, out_ap, meta):
    g_tiles = meta["g_tiles"]
    fins = meta["fins"]
    use_bout = meta["use_bout"]
    t_all = sum(g_tiles)
    rg = [list(range(NCORES))]

    res = ctx.enter_context(tc.tile_pool(name="res", bufs=1))
    wk = ctx.enter_context(tc.tile_pool(name="wk", bufs=4))
    hd = ctx.enter_context(tc.tile_pool(name="hd", bufs=1))
    mm = ctx.enter_context(tc.tile_pool(name="mm", bufs=3, space="PSUM"))
    acc = ctx.enter_context(tc.tile_pool(name="acc", bufs=2, space="PSUM"))
    hold = ctx.enter_context(tc.tile_pool(name="hold", bufs=1, space="PSUM"))
    dram = ctx.enter_context(tc.tile_pool(name="dram", bufs=1, space="DRAM"))

    def load(name, dt=None, pool=res):
        ap = io[name]
        t = pool.tile(list(ap.shape), dt or ap.dtype, name=f"L_{name}")
        nc.sync.dma_start(t[:], ap[:])
        return t

    # ---- residents ----
    srcb = load("srcidx")
    dcolb = load("dcol")
    maskb = load("maskbuf")
    iota_row = load("iota_row")
    iota_col = load("iota_col")
    ident = load("ident")
    identf32 = load("identf32")
    h0T = load("h0T")
    wts = {}
    for li in range(3):
        for k in range((fins[li] + P - 1) // P):
            wts[f"wl{li}k{k}"] = load(f"wl{li}k{k}")
            wts[f"wr{li}k{k}"] = load(f"wr{li}k{k}")
        for nm in (f"bl{li}", f"br{li}", f"att08_{li}"):
            wts[nm] = load(nm)
        if use_bout:
            wts[f"bout{li}"] = load(f"bout{li}")

    xr_buf = res.tile([P, NG * HC], F16, name="xr_buf")
    hT_buf = [res.tile([P, NPC], F16, name=f"hT{k}") for k in range(2)]

    ag_in = dram.tile([NPC, HCA], F16, name="ag_in")
    _shr = "Local" if "nocoll" in DBG_SKIP else "Shared"
    tables = [dram.tile([NPAD, HCA], F16, name=f"table{li}", addr_space=_shr)
              for li in range(3)]
    bn_in = dram.tile([2, HC], F32, name="bn_in")
    bn_out = dram.tile([2, HC], F32, name="bn_out", addr_space=_shr)
    outb = dram.tile([NPC, HC], F32, name="outb")
    pool_in = dram.tile([3 * P, HC], F32, name="pool_in")
    pool_out = dram.tile([3 * P, HC], F32, name="pool_out", addr_space=_shr)

    # ---------------- helpers ----------------
    def silu_(dst, src, pool=None):
        """dst = src * sigmoid(src)"""
        s = src[:]
        d = dst[:]
        sg = (pool or wk).tile([P, s.shape[1]], F32, name="sg", tag="sg")
        nc.scalar.activation(sg[:], s, mybir.ActivationFunctionType.Sigmoid)
        nc.vector.tensor_tensor(out=d, in0=s, in1=sg[:],
                                op=mybir.AluOpType.mult)

    def transpose128(dst, src_ap, n_out_part=P, n_in_part=P, f32=False):
        """dst[j, i] = src[i, j] via PE; src [n_in_part, n_out_part]"""
        tp = mm.tile([P, P], F32 if f32 else F16, name="mm_t", tag="mmshared")
        nc.tensor.transpose(out=tp[:n_out_part, :n_in_part], in_=src_ap,
                            identity=(identf32 if f32 else ident)[:])
        nc.scalar.copy(dst, tp[:n_out_part, :n_in_part])

    # ---------------- GAT layers ----------------
    for li in range(3):
        fin = fins[li]
        nkc = (fin + P - 1) // P  # weight K chunks
        bl, br = wts[f"bl{li}"], wts[f"br{li}"]
        att08 = wts[f"att08_{li}"]

        # ---- node phase: xl/xr = h @ W^T + b ----
        for g in range(NG):
            for side in range(2):
                b = bl if side == 0 else br
                wid = HCA if side == 0 else HC
                ps = mm.tile([P, 512], F32, name="mm_n", tag="mmshared")
                for k in range(nkc):
                    rows = min(P, fin - k * P)
                    w = wts[f"wl{li}k{k}"] if side == 0 else wts[f"wr{li}k{k}"]
                    if li == 0:
                        lhs = h0T[k * P:k * P + rows, g * P:(g + 1) * P]
                    else:
                        lhs = hT_buf[k][:rows, g * P:(g + 1) * P]
                    nc.tensor.matmul(ps[:, :wid], lhsT=lhs, rhs=w[:rows, :],
                                     start=(k == 0), stop=(k == nkc - 1))
                if side == 0:
                    xt = wk.tile([P, HCA], F16, name="xt")
                    nc.vector.tensor_tensor(out=xt[:], in0=ps[:, :HCA], in1=b[:],
                                            op=mybir.AluOpType.add)
                    nc.sync.dma_start(ag_in[g * P:(g + 1) * P, :], xt[:])
                else:
                    nc.vector.tensor_tensor(
                        out=xr_buf[:, g * HC:(g + 1) * HC], in0=ps[:, :HC],
                        in1=b[:], op=mybir.AluOpType.add)

        # ---- all-gather the table ----
        table = tables[li]
        if "nocoll" in DBG_SKIP:
            nc.sync.dma_start(table[0:NPC, :], ag_in[:])
        else:
            nc.gpsimd.collective_compute(
                "AllGather", mybir.AluOpType.bypass, replica_groups=rg,
                ins=[ag_in[:].opt()], outs=[table[:].opt()])

        # ---- edge phase ----
        if li == 2:
            spoolb = load("spool")
            pacc = hold.tile([P, HC], F32, name="pacc", space="PSUM")
        t0 = 0
        for g in range(NG):
            nt = g_tiles[g]
            gacc = acc.tile([P, HCA], F32, name="gacc", space="PSUM")
            t = 0
            while t < nt:
                B = 2 if t + 1 < nt else 1
                ti = t0 + t
                BW = B * HC
                # gathers into [P, B, HCA]
                G = wk.tile([P, 2 * HCA], F16, name="Gt")
                G3 = G[:].rearrange("p (b w) -> p b w", w=HCA)
                for bi in range(B):
                    nc.gpsimd.indirect_dma_start(
                        out=G3[:, bi, :], out_offset=None, in_=table[:],
                        in_offset=bass.IndirectOffsetOnAxis(
                            ap=srcb[:, ti + bi:ti + bi + 1], axis=0))
                # selection matrices (batched is_equal)
                ST = wk.tile([P, 2 * P], F16, name="ST")
                nc.vector.tensor_tensor(
                    out=ST[:, :B * P].rearrange("p (b n) -> p b n", b=B),
                    in0=dcolb[:, ti:ti + B][:, :, None].to_broadcast([P, B, P]),
                    in1=iota_row[:, None, :].to_broadcast([P, B, P]),
                    op=mybir.AluOpType.is_equal)
                S = wk.tile([P, 2 * P], F16, name="St")
                for bi in range(B):
                    Sp = mm.tile([P, P], F16, name="mm_s", tag="mmshared")
                    nc.tensor.transpose(out=Sp[:], in_=ST[:, bi * P:(bi + 1) * P],
                                        identity=ident[:])
                    nc.scalar.copy(S[:, bi * P:(bi + 1) * P], Sp[:])
                # Z = G_xl + S @ xr  accumulated in PSUM via identity-matmul
                zps = mm.tile([P, 512], F32, name="mm_z", tag="mmshared")
                for bi in range(B):
                    nc.tensor.matmul(zps[:, bi * HC:(bi + 1) * HC],
                                     lhsT=ident[:], rhs=G3[:, bi, :HC],
                                     start=True, stop=False)
                    nc.tensor.matmul(zps[:, bi * HC:(bi + 1) * HC],
                                     lhsT=S[:, bi * P:(bi + 1) * P],
                                     rhs=xr_buf[:, g * HC:(g + 1) * HC],
                                     start=False, stop=True)
                # r = relu(Z) from PSUM
                r = wk.tile([P, 2 * HC], F16, name="rt")
                nc.scalar.activation(r[:, :BW], zps[:, :BW],
                                     mybir.ActivationFunctionType.Relu)
                # ra = r * att08 ; lsum = reduce per head
                ra = wk.tile([P, 2 * HC], F16, name="rat")
                nc.vector.tensor_tensor(
                    out=ra[:, :BW].rearrange("p (b h c) -> p b h c", b=B, h=H),
                    in0=r[:, :BW].rearrange("p (b h c) -> p b h c", b=B, h=H),
                    in1=att08[:].rearrange("p (h c) -> p h c", h=H)[:, None, :, :]
                        .to_broadcast([P, B, H, C]),
                    op=mybir.AluOpType.mult)
                lsum = wk.tile([P, 2 * H], F32, name="lsum")
                nc.vector.tensor_reduce(
                    out=lsum[:, :B * H],
                    in_=ra[:, :BW].rearrange("p (bh c) -> p bh c", c=C),
                    axis=mybir.AxisListType.X, op=mybir.AluOpType.add)
                # logit = lsum + 0.2*al[src] (table cols 256:260)
                logit = wk.tile([P, 2 * H], F32, name="logit")
                nc.vector.tensor_tensor(
                    out=logit[:, :B * H].rearrange("p (b h) -> p b h", b=B),
                    in0=lsum[:, :B * H].rearrange("p (b h) -> p b h", b=B),
                    in1=G3[:, :B, HC:HCA], op=mybir.AluOpType.add)
                # p = exp(logit) -> pay[:, :, 256:260]; pay = [G*p | p]
                pay = wk.tile([P, 2 * HCA], F16, name="pay")
                pay3 = pay[:].rearrange("p (b w) -> p b w", w=HCA)
                nc.scalar.activation(
                    pay3[:, :B, HC:HCA],
                    logit[:, :B * H].rearrange("p (b h) -> p b h", b=B),
                    mybir.ActivationFunctionType.Exp)
                nc.vector.tensor_tensor(
                    out=pay3[:, :B, :HC].rearrange("p b (h c) -> p b h c", h=H),
                    in0=G3[:, :B, :HC].rearrange("p b (h c) -> p b h c", h=H),
                    in1=pay3[:, :B, HC:HCA][:, :, :, None].to_broadcast([P, B, H, C]),
                    op=mybir.AluOpType.mult)
                # scatter-accumulate into group PSUM
                for bi in range(B):
                    nc.tensor.matmul(gacc[:], lhsT=ST[:, bi * P:(bi + 1) * P],
                                     rhs=pay3[:, bi, :],
                                     start=(t + bi == 0), stop=(t + bi == nt - 1))
                t += B
            t0 += nt

            # ---- group finalize ----
            den = wk.tile([P, H], F32, name="den")
            nc.vector.tensor_scalar_add(den[:], gacc[:, HC:HCA], 1e-16)
            rec = wk.tile([P, H], F32, name="rec")
            nc.vector.reciprocal(rec[:], den[:])
            ogt = wk.tile([P, HC], F32, name="ogt")
            og = ogt[:, :]
            nc.vector.tensor_tensor(
                out=og.rearrange("p (h c) -> p h c", h=H),
                in0=gacc[:, :HC].rearrange("p (h c) -> p h c", h=H),
                in1=rec[:, :, None].to_broadcast([P, H, C]),
                op=mybir.AluOpType.mult)
            if use_bout:
                nc.vector.tensor_tensor(out=og, in0=og, in1=wts[f"bout{li}"][:],
                                        op=mybir.AluOpType.add)
            if li == 0:
                nc.sync.dma_start(outb[g * P:(g + 1) * P, :], ogt[:])
            elif li == 1:
                hg16 = wk.tile([P, HC], F16, name="hg16b")
                silu_(hg16, ogt)
                for k in range(2):
                    transpose128(hT_buf[k][:, g * P:(g + 1) * P],
                                 hg16[:, k * P:(k + 1) * P])
            else:
                hg16 = wk.tile([P, HC], F16, name="hg16c")
                silu_(hg16, ogt)
                nc.tensor.matmul(pacc[:], lhsT=spoolb[:, g * P:(g + 1) * P],
                                 rhs=hg16[:], start=(g == 0), stop=(g == NG - 1))

        # ---- layer activation ----
        if li == 0:
            # BN stats: s1/s2 via mask-weighted column-sum matmuls
            stats = hold.tile([33, HC], F32, name="stats", space="PSUM")
            for g in range(NG):
                ld = wk.tile([P, HC], F32, name="ld1")
                nc.sync.dma_start(ld[:], outb[g * P:(g + 1) * P, :])
                sq = wk.tile([P, HC], F32, name="sq")
                nc.scalar.activation(sq[:], ld[:], mybir.ActivationFunctionType.Square)
                nc.tensor.matmul(stats[0:1, :], lhsT=maskb[:, g:g + 1], rhs=ld[:],
                                 start=(g == 0), stop=(g == NG - 1))
                nc.tensor.matmul(stats[32:33, :], lhsT=maskb[:, g:g + 1], rhs=sq[:],
                                 start=(g == 0), stop=(g == NG - 1))
            s1_sb = hd.tile([1, HC], F32, name="s1_sb")
            s2_sb = hd.tile([1, HC], F32, name="s2_sb")
            nc.scalar.copy(s1_sb[:], stats[0:1, :])
            nc.scalar.copy(s2_sb[:], stats[32:33, :])
            nc.sync.dma_start(bn_in[0:1, :], s1_sb[:])
            nc.sync.dma_start(bn_in[1:2, :], s2_sb[:])
            nc.gpsimd.collective_compute(
                "AllReduce", mybir.AluOpType.add, replica_groups=rg,
                ins=[bn_in[:].opt()], outs=[bn_out[:].opt()])
            s1t = hd.tile([1, HC], F32, name="s1t")
            s2t = hd.tile([1, HC], F32, name="s2t")
            nc.sync.dma_start(s1t[:], bn_out[0:1, :])
            nc.sync.dma_start(s2t[:], bn_out[1:2, :])
            gam = load("bn_g", pool=hd)
            bet = load("bn_b", pool=hd)
            mu = hd.tile([1, HC], F32, name="mu")
            nc.vector.tensor_scalar_mul(mu[:], s1t[:], 1.0 / N_NODES)
            mu2 = hd.tile([1, HC], F32, name="mu2")
            nc.scalar.activation(mu2[:], mu[:], mybir.ActivationFunctionType.Square)
            var = hd.tile([1, HC], F32, name="var")
            nc.vector.tensor_scalar(var[:], s2t[:], 1.0 / N_NODES, None,
                                    op0=mybir.AluOpType.mult)
            nc.vector.tensor_tensor(out=var[:], in0=var[:], in1=mu2[:],
                                    op=mybir.AluOpType.subtract)
            nc.vector.tensor_scalar_add(var[:], var[:], EPS_BN)
            sd = hd.tile([1, HC], F32, name="sd")
            nc.scalar.activation(sd[:], var[:], mybir.ActivationFunctionType.Sqrt)
            rsd = hd.tile([1, HC], F32, name="rsd")
            nc.vector.reciprocal(rsd[:], sd[:])
            ge = hd.tile([1, HC], F32, name="ge")
            nc.vector.tensor_tensor(out=ge[:], in0=gam[:], in1=rsd[:],
                                    op=mybir.AluOpType.mult)
            muge = hd.tile([1, HC], F32, name="muge")
            nc.vector.tensor_tensor(out=muge[:], in0=mu[:], in1=ge[:],
                                    op=mybir.AluOpType.mult)
            be = hd.tile([1, HC], F32, name="be")
            nc.vector.tensor_tensor(out=be[:], in0=bet[:], in1=muge[:],
                                    op=mybir.AluOpType.subtract)
            ge_b = res.tile([P, HC], F32, name="ge_b")
            be_b = res.tile([P, HC], F32, name="be_b")
            if "pbcast" not in DBG_SKIP:
                nc.gpsimd.partition_broadcast(ge_b[:], ge[:])
                nc.gpsimd.partition_broadcast(be_b[:], be[:])
            else:
                nc.gpsimd.memset(ge_b[:], 1.0)
                nc.gpsimd.memset(be_b[:], 0.0)
            # second pass: h = silu(out*ge + be), transpose into hT
            for g in range(NG):
                ld = wk.tile([P, HC], F32, name="ld2")
                nc.sync.dma_start(ld[:], outb[g * P:(g + 1) * P, :])
                t1 = wk.tile([P, HC], F32, name="t1")
                nc.vector.tensor_tensor(out=t1[:], in0=ld[:], in1=ge_b[:],
                                        op=mybir.AluOpType.mult)
                nc.vector.tensor_tensor(out=t1[:], in0=t1[:], in1=be_b[:],
                                        op=mybir.AluOpType.add)
                hg16 = wk.tile([P, HC], F16, name="hg16")
                silu_(hg16, t1)
                for k in range(2):
                    transpose128(hT_buf[k][:, g * P:(g + 1) * P],
                                 hg16[:, k * P:(k + 1) * P])
        elif li == 2:
            prow = load("poolrow", pool=hd)
            psb = hd.tile([P, HC], F32, name="psb")
            nc.scalar.copy(psb[:], pacc[:])
            zz = hd.tile([P, HC], F32, name="zz")
            nc.gpsimd.memset(zz[:], 0.0)
            for k in range(3):
                nc.sync.dma_start(pool_in[k * P:(k + 1) * P, :], zz[:])
            nc.gpsimd.indirect_dma_start(
                out=pool_in[:], out_offset=bass.IndirectOffsetOnAxis(
                    ap=prow[:, :1], axis=0),
                in_=psb[:], in_offset=None)
            if "nocoll" in DBG_SKIP:
                ptmp = wk.tile([P, HC], F32, name="ptmp")
                for kk in range(3):
                    nc.sync.dma_start(ptmp[:], pool_in[kk * P:(kk + 1) * P, :])
                    nc.sync.dma_start(pool_out[kk * P:(kk + 1) * P, :], ptmp[:])
            else:
                nc.gpsimd.collective_compute(
                    "AllReduce", mybir.AluOpType.add, replica_groups=rg,
                    ins=[pool_in[:].opt()], outs=[pool_out[:].opt()])

    # ---------------- head (replicated fp32) ----------------
    ones = load("ones_col")

    # mol path: hm1 = mol @ W0m^T + b0m   [256, 64]
    molT = [load("molT_a"), load("molT_b")]
    w0m = [load("w0m_a"), load("w0m_b")]
    b0m = load("b0m", pool=hd)
    hm1 = [hd.tile([P, C], F32, name=f"hm1_{r}", tag="hm", bufs=6) for r in range(2)]
    for r in range(2):
        ps = mm.tile([P, 512], F32, name="mm_h", tag="mmshared")
        for k in range(2):
            nc.tensor.matmul(ps[:, :C], lhsT=molT[k][:, r * P:(r + 1) * P],
                             rhs=w0m[k][:], start=(k == 0), stop=(k == 1))
        nc.vector.tensor_tensor(out=hm1[r][:], in0=ps[:, :C], in1=b0m[:],
                                op=mybir.AluOpType.add)
    # BN over graphs
    mst = hold.tile([33, C], F32, name="mst", space="PSUM")
    for r in range(2):
        sq = wk.tile([P, C], F32, name="msq")
        nc.scalar.activation(sq[:], hm1[r][:], mybir.ActivationFunctionType.Square)
        nc.tensor.matmul(mst[0:1, :], lhsT=ones[:], rhs=hm1[r][:],
                         start=(r == 0), stop=(r == 1))
        nc.tensor.matmul(mst[32:33, :], lhsT=ones[:], rhs=sq[:],
                         start=(r == 0), stop=(r == 1))
    gm = load("bnm_g", pool=hd)
    bm = load("bnm_b", pool=hd)
    mu = hd.tile([1, C], F32, name="mmu")
    nc.vector.tensor_scalar_mul(mu[:], mst[0:1, :], 1.0 / N_GRAPHS)
    mu2 = hd.tile([1, C], F32, name="mmu2")
    nc.scalar.activation(mu2[:], mu[:], mybir.ActivationFunctionType.Square)
    var = hd.tile([1, C], F32, name="mvar")
    nc.vector.tensor_scalar(var[:], mst[32:33, :], 1.0 / N_GRAPHS, None,
                            op0=mybir.AluOpType.mult)
    nc.vector.tensor_tensor(out=var[:], in0=var[:], in1=mu2[:],
                            op=mybir.AluOpType.subtract)
    nc.vector.tensor_scalar_add(var[:], var[:], EPS_BN)
    sd = hd.tile([1, C], F32, name="msd")
    nc.scalar.activation(sd[:], var[:], mybir.ActivationFunctionType.Sqrt)
    rsd = hd.tile([1, C], F32, name="mrsd")
    nc.vector.reciprocal(rsd[:], sd[:])
    ge = hd.tile([1, C], F32, name="mge")
    nc.vector.tensor_tensor(out=ge[:], in0=gm[:], in1=rsd[:],
                            op=mybir.AluOpType.mult)
    muge = hd.tile([1, C], F32, name="mmuge")
    nc.vector.tensor_tensor(out=muge[:], in0=mu[:], in1=ge[:],
                            op=mybir.AluOpType.mult)
    be = hd.tile([1, C], F32, name="mbe")
    nc.vector.tensor_tensor(out=be[:], in0=bm[:], in1=muge[:],
                            op=mybir.AluOpType.subtract)
    ge_b = hd.tile([P, C], F32, name="mge_b")
    be_b = hd.tile([P, C], F32, name="mbe_b")
    nc.gpsimd.partition_broadcast(ge_b[:], ge[:])
    nc.gpsimd.partition_broadcast(be_b[:], be[:])
    hm2 = [hd.tile([P, C], F32, name=f"hm2_{r}", tag="hm", bufs=6) for r in range(2)]
    for r in range(2):
        nc.vector.tensor_tensor(out=hm2[r][:], in0=hm1[r][:], in1=ge_b[:],
                                op=mybir.AluOpType.mult)
        nc.vector.tensor_tensor(out=hm2[r][:], in0=hm2[r][:], in1=be_b[:],
                                op=mybir.AluOpType.add)
        silu_(hm2[r], hm2[r], pool=hd)  # f32 in-place via sigmoid+mul (dst f32)
    # hm3 = silu(hm2 @ W1m^T + b1m)
    hm2T = hd.tile([C, N_GRAPHS], F32, name="hm2T")
    for r in range(2):
        transpose128(hm2T[:, r * P:(r + 1) * P], hm2[r][:, :C], n_out_part=C,
                     f32=True)
    w1m = load("w1mT", pool=hd)
    b1m = load("b1m", pool=hd)
    hm3 = [hd.tile([P, C], F32, name=f"hm3_{r}", tag="hm", bufs=6) for r in range(2)]
    for r in range(2):
        ps = mm.tile([P, 512], F32, name="mm_h2", tag="mmshared")
        nc.tensor.matmul(ps[:, :C], lhsT=hm2T[:, r * P:(r + 1) * P], rhs=w1m[:],
                         start=True, stop=True)
        nc.vector.tensor_tensor(out=hm3[r][:], in0=ps[:, :C], in1=b1m[:],
                                op=mybir.AluOpType.add)
        silu_(hm3[r], hm3[r], pool=hd)

    # concat [hg | hm3] -> [256, 320]
    cur = [hd.tile([P, 512], F32, name=f"hgc_{r}", tag="ioh", bufs=4) for r in range(2)]
    for r in range(2):
        nc.sync.dma_start(cur[r][:, :HC], pool_out[r * P:(r + 1) * P, :])
        nc.vector.tensor_copy(cur[r][:, HC:HC + C], hm3[r][:])

    # fc chain
    fc_dims = [(HC + C, 512), (512, 512), (512, 256), (256, 1)]
    for i, (fin, fout) in enumerate(fc_dims):
        nk = (fin + P - 1) // P
        curT = [hd.tile([min(P, fin - k * P), N_GRAPHS], F32,
                        name=f"curT{i}_{k}", tag="curT", bufs=4) for k in range(nk)]
        for r in range(2):
            for k in range(nk):
                rows = min(P, fin - k * P)
                transpose128(curT[k][:, r * P:(r + 1) * P][:, :],
                             cur[r][:, k * P:k * P + rows], n_out_part=rows,
                             f32=True)
        wch = [hd.tile([io[f"fc{i}w{k}"].shape[0], io[f"fc{i}w{k}"].shape[1]], F32, name=f"Lfc{i}w{k}", tag="fcw", bufs=4) for k in range(nk)]
        for k in range(nk):
            nc.sync.dma_start(wch[k][:], io[f"fc{i}w{k}"][:])
        bch = hd.tile([P, fc_dims[i][1]], F32, name=f"Lfc{i}b", tag="fcb", bufs=2)
        nc.sync.dma_start(bch[:], io[f"fc{i}b"][:])
        nxt = [hd.tile([P, max(fout, 1)], F32, name=f"nx{i}_{r}", tag="ioh", bufs=4) for r in range(2)]
        for r in range(2):
            ps = mm.tile([P, 512], F32, name=f"mm_fc", tag="mmshared")
            for k in range(nk):
                rows = min(P, fin - k * P)
                nc.tensor.matmul(ps[:, :fout],
                                 lhsT=curT[k][:, r * P:(r + 1) * P],
                                 rhs=wch[k][:], start=(k == 0), stop=(k == nk - 1))
            nc.vector.tensor_tensor(out=nxt[r][:, :fout], in0=ps[:, :fout],
                                    in1=bch[:, :fout], op=mybir.AluOpType.add)
            if i < 3:
                silu_(nxt[r], nxt[r], pool=hd)
        cur = nxt

    for r in range(2):
        nc.sync.dma_start(out_ap[r * P:(r + 1) * P, :], cur[r][:, :1])


# ----------------------------------------------------------------------------
# public entry
# ----------------------------------------------------------------------------

def get_program_and_inputs(x, edge_index, batch, mol_feats, params):
    in_maps, meta = _prep_inputs(x, edge_index, batch, mol_feats, params)
    key = (meta["g_tiles"], meta["use_bout"])
    if key not in _CACHE:
        _CACHE[key] = _build_program(meta)
    return _CACHE[key], in_maps


def kernel(x, edge_index, batch, mol_feats, params):
    nc, in_maps = get_program_and_inputs(x, edge_index, batch, mol_feats, params)
    res = run_bass_kernel_spmd(nc, in_maps, list(range(NCORES)))
    return np.asarray(res.results[0]["out"], dtype=np.float32)
